# revision 6
# baseline (speedup 1.0000x reference)
"""Trainium2 Bass kernel for nn_BaconAdditionReasoner (histogram_binning).

Math (per batch row):
    P1 = soft_perm(W1), P2 = soft_perm(W2)           (host, 10x10)
    l1 = p1 @ P1.T, l2 = p2 @ P2.T
    u = log(1-l1), v = log(1-l2)
    logprod[k] = sum_{i+j=k} max(u_i, v_j)
              = sum_{i+j=k} u_i + sum_{i+j=k} relu(v_j - u_i)
    e = exp(logprod);  out_k = (e_k - 1) / (sum_k e_k - 19)

Device dataflow (data parallel over 8 cores, 32768 rows/core):
  Front is feature-major: 4 bands of 20 feature rows at 32-aligned
  partitions (PE tile_position requires 32-aligned moving bases), batch on
  the free dim; 8 supertiles of 4 bands x 1024 cols = 4096 rows.
  - L matmul: one blockdiag [116->116] f16 matmul per supertile
  - Ln (ACT): uv = log(1 - l), f16
  - D matmuls: per band, [20->110] f16 pair-diff expansion (v_j - u_i,
    plus -u passthrough rows)
  - relu: split across ACT / DVE / Pool by column ranges (tunable)
  - A-flip matmuls: per 128-col block, kt [110, 128] is loaded as the
    STATIONARY operand and a tiny [110, 19] +-1 matrix streams as the
    moving operand (19 cycles/block, LDWEIGHTS is free) -> batch-major
    logprod [128 rows, 19 k's] in PSUM
  - Exp (ACT) -> f32, per-row 19-group reduce / recip / (e-1)*r on DVE
  - output dumped partition-major [128, 4864] f16; host de-interleaves.

All HBM I/O and matmul moving operands are f16 (validated on the real
input distribution: max rel err ~2.7e-3 vs the 2e-2 gate).
"""

import numpy as np

# ---------------------------------------------------------------- constants
B = 262144
NCORES = 8
BC = B // NCORES            # 32768 rows per core
F = 1024                    # batch columns per supertile (per band)
NB = 4                      # bands (32-aligned partition offsets)
ROWS_ST = F * NB            # 4096 rows per supertile
NST = BC // ROWS_ST         # 8 supertiles per core
NCOLS = NST * F             # 8192 columns in pc
NBLK = ROWS_ST // 128       # 32 A-flip blocks per supertile
KC = 19 * NBLK              # 608 output cols per supertile
OCOLS = KC * NST            # 4864 output cols

# wk (constants, f16 [128, 256]) column layout
WL0, WL1 = 0, 116           # L blockdiag lhsT [116, 116]
WD0, WD1 = 116, 226         # D pair lhsT [20, 110] replicated per band
WA0, WA1 = 226, 245         # A-flip moving [110, 19]
WKC = 256                   # padded so DMA elem = 512 B

# relu column split per dp tile q: list of (engine, c0, c1)
# engines: "A" = ACT (scalar), "D" = DVE (vector), "P" = Pool (gpsimd)
# band 0 split across two engines for low latency (A-flip blocks of band 0
# are first in PE program order); Pool takes the latency-tolerant tail.
RELU_SCHEME = [
    [("D", 0, 512), ("A", 512, F)],
    [("D", 0, F)],
    [("A", 0, 256), ("P", 256, F)],
    [("P", 0, F)],
]


def _soft_perm_np(W: np.ndarray) -> np.ndarray:
    W = W.astype(np.float32)
    lo = W.min(axis=1, keepdims=True)
    hi = W.max(axis=1, keepdims=True)
    Wn = (W - lo) / (hi - lo + np.float32(1e-8))
    return Wn / (Wn.sum(axis=1, keepdims=True) + np.float32(1e-8))


def _build_wk(P1n: np.ndarray, P2n: np.ndarray) -> np.ndarray:
    wk = np.zeros((128, WKC), dtype=np.float32)
    # --- L: lhsT[32q+d, 32q+e] = PP[e, d], PP = blockdiag(P1n, P2n)
    for q in range(NB):
        r = 32 * q
        wk[r : r + 10, r : r + 10] = P1n.T
        wk[r + 10 : r + 20, r + 10 : r + 20] = P2n.T
    # --- D: [20, 110]: pair col 10i+j gets v_j - u_i; col 100+e gets -u_e
    d = np.zeros((20, 110), dtype=np.float32)
    for i in range(10):
        for j in range(10):
            d[i, 10 * i + j] = -1.0
            d[10 + j, 10 * i + j] = 1.0
    for e in range(10):
        d[e, 100 + e] = -1.0
    for q in range(NB):
        wk[32 * q : 32 * q + 20, WD0:WD1] = d
    # --- A-flip moving [110, 19]: pair rows +1 at k=i+j; passthrough rows
    #     (-u values) -1 for k in [e, e+9]
    a = np.zeros((110, 19), dtype=np.float32)
    for i in range(10):
        for j in range(10):
            a[10 * i + j, i + j] = 1.0
    for e in range(10):
        a[100 + e, e : e + 10] = -1.0
    wk[0:110, WA0:WA1] = a
    return wk.astype(np.float16)


def _build_pc(p1c: np.ndarray, p2c: np.ndarray) -> np.ndarray:
    """[BC,10]x2 -> pc [116, NCOLS] f16: row 32q+e = feature e (u: e<10,
    v: 10<=e<20) of band q; col F*s+f = batch row ROWS_ST*s + F*q + f."""
    pc = np.zeros((116, NCOLS), dtype=np.float16)
    x1 = p1c.reshape(NST, NB, F, 10)    # [s, q, f, d]
    x2 = p2c.reshape(NST, NB, F, 10)
    for q in range(NB):
        pc[32 * q : 32 * q + 10, :] = (
            x1[:, q].transpose(2, 0, 1).reshape(10, NCOLS).astype(np.float16)
        )
        pc[32 * q + 10 : 32 * q + 20, :] = (
            x2[:, q].transpose(2, 0, 1).reshape(10, NCOLS).astype(np.float16)
        )
    return pc


def _unpack_yraw(yraw: np.ndarray) -> np.ndarray:
    """yraw [128, OCOLS] f16 -> y [BC, 19] f32.
    yraw[p, KC*s + 19*b + k] = y[ROWS_ST*s + 128*b + p, k]."""
    t = yraw.reshape(128, NST, NBLK, 19).transpose(1, 2, 0, 3)
    return np.ascontiguousarray(t.reshape(BC, 19).astype(np.float32))


def _patch_act_tables():
    """Force Ln/Exp/Relu to resolve to the single set containing all three
    (natural_log_exp_and_others) so the activation table is loaded once."""
    import concourse.bacc as bacc
    from concourse import mybir

    if getattr(bacc, "_act_tables_patched", False):
        return
    orig = bacc.get_activation_tables
    AF = mybir.ActivationFunctionType
    shared = {AF.Ln, AF.Exp, AF.Relu}

    def patched(arch):
        tabs = orig(arch)
        if "natural_log_exp_and_others" in tabs:
            for name, funcs in tabs.items():
                if name != "natural_log_exp_and_others":
                    tabs[name] = set(funcs) - shared
        return tabs

    bacc.get_activation_tables = patched
    bacc._act_tables_patched = True


def build_bass():
    import concourse.bass as bass
    import concourse.bacc as bacc
    import concourse.tile as tile
    from concourse import mybir

    _patch_act_tables()
    f32 = mybir.dt.float32
    f16 = mybir.dt.float16
    AF = mybir.ActivationFunctionType
    ALU = mybir.AluOpType

    nc = bacc.Bacc("TRN2", target_bir_lowering=False)

    pc_d = nc.dram_tensor("pc", [116, NCOLS], f16, kind="ExternalInput")
    wk_d = nc.dram_tensor("wk", [128, WKC], f16, kind="ExternalInput")
    y_d = nc.dram_tensor("yraw", [128, OCOLS], f16, kind="ExternalOutput")

    with tile.TileContext(nc) as tc:
        with (
            tc.tile_pool(name="singles", bufs=1) as singles,
            tc.tile_pool(name="uv", bufs=2) as uv_p,
            tc.tile_pool(name="kt", bufs=2) as kt_p,
            tc.tile_pool(name="ee", bufs=2) as ee_p,
            tc.tile_pool(name="ss", bufs=2) as ss_p,
            tc.tile_pool(name="rr", bufs=2) as rr_p,
            tc.tile_pool(name="psL", bufs=1, space="PSUM") as psL,
            tc.tile_pool(name="psD", bufs=2, space="PSUM") as psD,
            tc.tile_pool(name="psA", bufs=1, space="PSUM") as psA,
        ):
            wk = singles.tile([128, WKC], f16)
            nc.sync.dma_start(wk[:, :], wk_d[:, :])
            oo = singles.tile([128, OCOLS], f16)

            # prefetch the whole input up front (16 KB/partition), chunked so
            # supertile 0 can start after the first quarter lands
            pcc = singles.tile([116, NCOLS], f16)
            for c in range(4):
                c0 = c * (NCOLS // 4)
                nc.sync.dma_start(
                    pcc[:, c0 : c0 + NCOLS // 4], pc_d[:, c0 : c0 + NCOLS // 4]
                )

            for s in range(NST):
                off = F * s

                # l = blockdiag(P1n, P2n) @ p  (one matmul, f16 moving)
                lp = psL.tile([116, F], f32)
                nc.tensor.matmul(
                    lp[:, :], wk[0:116, WL0:WL1], pcc[0:116, off : off + F],
                    start=True, stop=True,
                )
                # uv = log(1 - l)
                uvt = uv_p.tile([116, F], f16)
                nc.scalar.activation(
                    uvt[:, :], lp[:, :], AF.Ln, bias=1.0, scale=-1.0
                )

                ap_t = psA.tile([128, KC], f32)

                def emit_d(q):
                    r = 32 * q
                    dp = psD.tile([110, F], f32)
                    nc.tensor.matmul(
                        dp[:, :], wk[r : r + 20, WD0:WD1], uvt[r : r + 20, :],
                        start=True, stop=True, tile_position=(r, 0),
                    )
                    return dp

                def emit_relu(q, dp):
                    kt = kt_p.tile([110, F], f16)
                    for eng, a0, a1 in RELU_SCHEME[q]:
                        if eng == "A":
                            nc.scalar.activation(
                                kt[:, a0:a1], dp[:, a0:a1], AF.Relu
                            )
                        elif eng == "D":
                            nc.vector.tensor_scalar(
                                kt[:, a0:a1], dp[:, a0:a1], 0.0, None,
                                op0=ALU.max,
                            )
                        else:
                            nc.gpsimd.tensor_scalar(
                                kt[:, a0:a1], dp[:, a0:a1], 0.0, None,
                                op0=ALU.max,
                            )
                    return kt

                def emit_aflip(q, kt):
                    # batch-major logprod: kt block stationary, [110,19] moving
                    for b in range(F // 128):
                        blk = (F // 128) * q + b
                        nc.tensor.matmul(
                            ap_t[:, 19 * blk : 19 * blk + 19],
                            kt[0:110, 128 * b : 128 * b + 128],
                            wk[0:110, WA0:WA1],
                            start=True, stop=True,
                        )

                # PE program order: L D0 D1 [A0] D2 [A1] D3 [A2] [A3] so the
                # PE never parks on a relu that hasn't finished (psD bufs=2)
                dp0 = emit_d(0)
                kt0 = emit_relu(0, dp0)
                dp1 = emit_d(1)
                kt1 = emit_relu(1, dp1)
                emit_aflip(0, kt0)
                dp2 = emit_d(2)
                kt2 = emit_relu(2, dp2)
                emit_aflip(1, kt1)
                dp3 = emit_d(3)
                kt3 = emit_relu(3, dp3)
                emit_aflip(2, kt2)
                emit_aflip(3, kt3)

                # e = exp(logprod)  (f32: e-1 cancellation needs mantissa)
                e32 = ee_p.tile([128, KC], f32)
                nc.scalar.activation(e32[:, :], ap_t[:, :], AF.Exp)
                ev = e32[:, :].rearrange("p (b k) -> p b k", b=NBLK, k=19)
                s32 = ss_p.tile([128, NBLK], f32)
                nc.vector.tensor_reduce(
                    s32[:, :], ev, axis=mybir.AxisListType.X, op=ALU.add
                )
                sm = ss_p.tile([128, NBLK], f32)
                nc.vector.tensor_scalar(
                    sm[:, :], s32[:, :], -19.0, None, op0=ALU.add
                )
                r32 = rr_p.tile([128, NBLK], f32)
                nc.vector.reciprocal(r32[:, :], sm[:, :])
                # out = (e - 1) * r  ==  (1-e)/(19-sum(e)), f16
                ov = oo[:, KC * s : KC * (s + 1)].rearrange(
                    "p (b k) -> p b k", b=NBLK, k=19
                )
                rb = r32[:, :].unsqueeze(-1).broadcast_to([128, NBLK, 19])
                nc.vector.scalar_tensor_tensor(
                    ov, ev, 1.0, rb, op0=ALU.subtract, op1=ALU.mult
                )
                if s % 2 == 1:
                    o0 = KC * (s - 1)
                    nc.sync.dma_start(
                        y_d[:, o0 : o0 + 2 * KC], oo[:, o0 : o0 + 2 * KC]
                    )
    nc.compile()
    return nc


_NC_CACHE = None


def kernel(p1, p2, W1, W2):
    global _NC_CACHE
    from concourse.bass_utils import run_bass_kernel_spmd

    P1n = _soft_perm_np(np.asarray(W1))
    P2n = _soft_perm_np(np.asarray(W2))
    wk = _build_wk(P1n, P2n)
    p1 = np.ascontiguousarray(np.asarray(p1, dtype=np.float32))
    p2 = np.ascontiguousarray(np.asarray(p2, dtype=np.float32))

    in_maps = []
    for c in range(NCORES):
        sl = slice(c * BC, (c + 1) * BC)
        in_maps.append({"pc": _build_pc(p1[sl], p2[sl]), "wk": wk})

    if _NC_CACHE is None:
        _NC_CACHE = build_bass()
    res = run_bass_kernel_spmd(_NC_CACHE, in_maps, core_ids=list(range(NCORES)))
    out = np.concatenate(
        [_unpack_yraw(res.results[c]["yraw"]) for c in range(NCORES)], axis=0
    )
    return out


# revision 7
# speedup vs baseline: 1.2540x; 1.2540x over previous
"""Trainium2 Bass kernel for nn_BaconAdditionReasoner (histogram_binning).

Math (per batch row):
    P1 = soft_perm(W1), P2 = soft_perm(W2)           (host, 10x10)
    l1 = p1 @ P1.T, l2 = p2 @ P2.T
    u = log(1-l1), v = log(1-l2)
    logprod[k] = sum_{i+j=k} max(u_i, v_j)
              = sum_{i+j=k} u_i + sum_{i+j=k} relu(v_j - u_i)
    e = exp(logprod);  out_k = (e_k - 1) / (sum_k e_k - 19)

Device dataflow (data parallel over 8 cores, 32768 rows/core):
  Front is feature-major: 4 bands of 20 feature rows at 32-aligned
  partitions (PE tile_position needs 32-aligned moving bases), batch on the
  free dim, in half-supertiles of 512 cols (2048 rows); processed in PAIRS
  (1024 cols) so Ln / Exp / the normalization tail amortize their
  per-instruction SBUF/PSUM access cost.
  - L matmul: blockdiag [116->116] f16 matmul per half
  - Ln (ACT): uv = log(1 - l) -> f16, one instr per pair
  - D matmuls: per band, [20->110] f16 pair-diff expansion in PSUM
    (v_j - u_i pairs + -u passthrough rows); 4 PSUM bufs = all bands in
    flight, so one slow relu never starves the PE
  - relu: split across ACT / DVE / Pool per band tile (tunable scheme)
  - A-flip matmuls: per 128-col block, kt [110, 128] is the STATIONARY
    operand (LDWEIGHTS is free) and a [110, 19] +-1 matrix streams as the
    moving operand -> 19 cycles/block; batch-major logprod in PSUM
  - Exp (ACT) -> f32; 19-group reduce / recip / (e-1)*r on DVE per pair
  - output dumped partition-major [128, 4864] f16; host de-interleaves.

All HBM I/O and matmul moving operands are f16 (validated on the real
input distribution: max rel err ~2.7e-3 vs the 2e-2 gate).
"""

import numpy as np

# ---------------------------------------------------------------- constants
B = 262144
NCORES = 8
BC = B // NCORES            # 32768 rows per core
F = 512                     # batch columns per half-supertile (per band)
NB = 4                      # bands (32-aligned partition offsets)
ROWS_H = F * NB             # 2048 rows per half
NH = BC // ROWS_H           # 16 halves per core
NP = NH // 2                # 8 pairs
NCOLS = NH * F              # 8192 columns in pc
KCH = 19 * (ROWS_H // 128)  # 304 output cols per half
KC = 2 * KCH                # 608 per pair
OCOLS = KC * NP             # 4864 output cols

# wk (constants, f16 [128, 256]) column layout
WL0, WL1 = 0, 116           # L blockdiag lhsT [116, 116]
WD0, WD1 = 116, 226         # D pair lhsT [20, 110] replicated per band
WA0, WA1 = 226, 245         # A-flip moving [110, 19]
WKC = 256                   # padded so DMA elem = 512 B

# relu engine per band tile (8 tiles per pair = 2 halves x 4 bands), each a
# list of (engine, c0, c1) over [0, F).
# engines: "A" = ACT (scalar), "D" = DVE (vector), "P" = Pool (gpsimd)
RELU_SCHEME = [
    [("A", 0, F)], [("D", 0, F)], [("P", 0, F)], [("P", 0, F)],
    [("A", 0, F)], [("D", 0, F)], [("P", 0, F)], [("D", 0, F)],
]


def _soft_perm_np(W: np.ndarray) -> np.ndarray:
    W = W.astype(np.float32)
    lo = W.min(axis=1, keepdims=True)
    hi = W.max(axis=1, keepdims=True)
    Wn = (W - lo) / (hi - lo + np.float32(1e-8))
    return Wn / (Wn.sum(axis=1, keepdims=True) + np.float32(1e-8))


def _build_wk(P1n: np.ndarray, P2n: np.ndarray) -> np.ndarray:
    wk = np.zeros((128, WKC), dtype=np.float32)
    # --- L: lhsT[32q+d, 32q+e] = PP[e, d], PP = blockdiag(P1n, P2n)
    for q in range(NB):
        r = 32 * q
        wk[r : r + 10, r : r + 10] = P1n.T
        wk[r + 10 : r + 20, r + 10 : r + 20] = P2n.T
    # --- D: [20, 110]: pair col 10i+j gets v_j - u_i; col 100+e gets -u_e
    d = np.zeros((20, 110), dtype=np.float32)
    for i in range(10):
        for j in range(10):
            d[i, 10 * i + j] = -1.0
            d[10 + j, 10 * i + j] = 1.0
    for e in range(10):
        d[e, 100 + e] = -1.0
    for q in range(NB):
        wk[32 * q : 32 * q + 20, WD0:WD1] = d
    # --- A-flip moving [110, 19]: pair rows +1 at k=i+j; passthrough rows
    #     (-u values) -1 for k in [e, e+9]
    a = np.zeros((110, 19), dtype=np.float32)
    for i in range(10):
        for j in range(10):
            a[10 * i + j, i + j] = 1.0
    for e in range(10):
        a[100 + e, e : e + 10] = -1.0
    wk[0:110, WA0:WA1] = a
    return wk.astype(np.float16)


def _build_pc(p1c: np.ndarray, p2c: np.ndarray) -> np.ndarray:
    """[BC,10]x2 -> pc [116, NCOLS] f16: row 32q+e = feature e (u: e<10,
    v: 10<=e<20) of band q; col F*g+f = batch row ROWS_H*g + F*q + f."""
    pc = np.zeros((116, NCOLS), dtype=np.float16)
    x1 = p1c.reshape(NH, NB, F, 10)     # [g, q, f, d]
    x2 = p2c.reshape(NH, NB, F, 10)
    for q in range(NB):
        pc[32 * q : 32 * q + 10, :] = (
            x1[:, q].transpose(2, 0, 1).reshape(10, NCOLS).astype(np.float16)
        )
        pc[32 * q + 10 : 32 * q + 20, :] = (
            x2[:, q].transpose(2, 0, 1).reshape(10, NCOLS).astype(np.float16)
        )
    return pc


def _unpack_yraw(yraw: np.ndarray) -> np.ndarray:
    """yraw [128, OCOLS] f16 -> y [BC, 19] f32.
    yraw[p, KC*t + 19*(16h+4q+b) + k] = y[4096t+2048h+512q+128b+p, k]."""
    t = yraw.reshape(128, NP, 2, NB, 4, 19).transpose(1, 2, 3, 4, 0, 5)
    return np.ascontiguousarray(t.reshape(BC, 19).astype(np.float32))


def _patch_act_tables():
    """Force Ln/Exp/Relu to resolve to the single set containing all three
    (natural_log_exp_and_others) so the activation table is loaded once."""
    import concourse.bacc as bacc
    from concourse import mybir

    if getattr(bacc, "_act_tables_patched", False):
        return
    orig = bacc.get_activation_tables
    AF = mybir.ActivationFunctionType
    shared = {AF.Ln, AF.Exp, AF.Relu}

    def patched(arch):
        tabs = orig(arch)
        if "natural_log_exp_and_others" in tabs:
            for name, funcs in tabs.items():
                if name != "natural_log_exp_and_others":
                    tabs[name] = set(funcs) - shared
        return tabs

    bacc.get_activation_tables = patched
    bacc._act_tables_patched = True


def build_bass():
    import concourse.bass as bass
    import concourse.bacc as bacc
    import concourse.tile as tile
    from concourse import mybir

    _patch_act_tables()
    f32 = mybir.dt.float32
    f16 = mybir.dt.float16
    AF = mybir.ActivationFunctionType
    ALU = mybir.AluOpType

    nc = bacc.Bacc("TRN2", target_bir_lowering=False)

    pc_d = nc.dram_tensor("pc", [116, NCOLS], f16, kind="ExternalInput")
    wk_d = nc.dram_tensor("wk", [128, WKC], f16, kind="ExternalInput")
    y_d = nc.dram_tensor("yraw", [128, OCOLS], f16, kind="ExternalOutput")

    with tile.TileContext(nc) as tc:
        with (
            tc.tile_pool(name="singles", bufs=1) as singles,
            tc.tile_pool(name="uv", bufs=2) as uv_p,
            tc.tile_pool(name="kt", bufs=4) as kt_p,
            tc.tile_pool(name="ee", bufs=2) as ee_p,
            tc.tile_pool(name="ss", bufs=2) as ss_p,
            tc.tile_pool(name="rr", bufs=2) as rr_p,
            tc.tile_pool(name="psL", bufs=1, space="PSUM") as psL,
            tc.tile_pool(name="psD", bufs=4, space="PSUM") as psD,
            tc.tile_pool(name="psA", bufs=1, space="PSUM") as psA,
        ):
            wk = singles.tile([128, WKC], f16)
            nc.sync.dma_start(wk[:, :], wk_d[:, :])
            oo = singles.tile([128, OCOLS], f16)

            # prefetch the whole input up front (16 KB/partition), chunked so
            # pair 0 can start after the first quarter lands
            pcc = singles.tile([116, NCOLS], f16)
            for c in range(4):
                c0 = c * (NCOLS // 4)
                nc.sync.dma_start(
                    pcc[:, c0 : c0 + NCOLS // 4], pc_d[:, c0 : c0 + NCOLS // 4]
                )

            def emit_relu(scheme, dp):
                kt = kt_p.tile([110, F], f16)
                for eng, a0, a1 in scheme:
                    if eng == "A":
                        nc.scalar.activation(kt[:, a0:a1], dp[:, a0:a1], AF.Relu)
                    elif eng == "D":
                        nc.vector.tensor_scalar(
                            kt[:, a0:a1], dp[:, a0:a1], 0.0, None, op0=ALU.max
                        )
                    else:
                        nc.gpsimd.tensor_scalar(
                            kt[:, a0:a1], dp[:, a0:a1], 0.0, None, op0=ALU.max
                        )
                return kt

            for t in range(NP):
                base = 2 * F * t
                # l = blockdiag(P1n, P2n) @ p ; one Ln per pair
                lp = psL.tile([116, 2 * F], f32)
                for h in range(2):
                    nc.tensor.matmul(
                        lp[:, F * h : F * (h + 1)], wk[0:116, WL0:WL1],
                        pcc[0:116, base + F * h : base + F * (h + 1)],
                        start=True, stop=True,
                    )
                uvt = uv_p.tile([116, 2 * F], f16)
                nc.scalar.activation(
                    uvt[:, :], lp[:, :], AF.Ln, bias=1.0, scale=-1.0
                )

                ap_t = psA.tile([128, KC], f32)
                for h in range(2):
                    uh = uvt[:, F * h : F * (h + 1)]
                    dps = []
                    for q in range(NB):
                        r = 32 * q
                        dp = psD.tile([110, F], f32)
                        nc.tensor.matmul(
                            dp[:, :], wk[r : r + 20, WD0:WD1], uh[r : r + 20, :],
                            start=True, stop=True, tile_position=(r, 0),
                        )
                        dps.append(dp)
                    kts = [
                        emit_relu(RELU_SCHEME[4 * h + q], dps[q])
                        for q in range(NB)
                    ]
                    # batch-major logprod: kt block stationary, [110,19] moving
                    for q in range(NB):
                        for b in range(F // 128):
                            blk = 16 * h + 4 * q + b
                            nc.tensor.matmul(
                                ap_t[:, 19 * blk : 19 * blk + 19],
                                kts[q][0:110, 128 * b : 128 * b + 128],
                                wk[0:110, WA0:WA1],
                                start=True, stop=True,
                            )

                # e = exp(logprod) (f32: e-1 cancellation needs mantissa)
                e32 = ee_p.tile([128, KC], f32)
                nc.scalar.activation(e32[:, :], ap_t[:, :], AF.Exp)
                ev = e32[:, :].rearrange("p (b k) -> p b k", b=KC // 19, k=19)
                s32 = ss_p.tile([128, KC // 19], f32)
                nc.vector.tensor_reduce(
                    s32[:, :], ev, axis=mybir.AxisListType.X, op=ALU.add
                )
                sm = ss_p.tile([128, KC // 19], f32)
                nc.vector.tensor_scalar(
                    sm[:, :], s32[:, :], -19.0, None, op0=ALU.add
                )
                r32 = rr_p.tile([128, KC // 19], f32)
                nc.vector.reciprocal(r32[:, :], sm[:, :])
                # out = (e - 1) * r  ==  (1-e)/(19-sum(e)), f16
                ov = oo[:, KC * t : KC * (t + 1)].rearrange(
                    "p (b k) -> p b k", b=KC // 19, k=19
                )
                rb = r32[:, :].unsqueeze(-1).broadcast_to([128, KC // 19, 19])
                nc.vector.scalar_tensor_tensor(
                    ov, ev, 1.0, rb, op0=ALU.subtract, op1=ALU.mult
                )
                if t % 2 == 1:
                    o0 = KC * (t - 1)
                    nc.sync.dma_start(
                        y_d[:, o0 : o0 + 2 * KC], oo[:, o0 : o0 + 2 * KC]
                    )
    nc.compile()
    return nc


_NC_CACHE = None


def kernel(p1, p2, W1, W2):
    global _NC_CACHE
    from concourse.bass_utils import run_bass_kernel_spmd

    P1n = _soft_perm_np(np.asarray(W1))
    P2n = _soft_perm_np(np.asarray(W2))
    wk = _build_wk(P1n, P2n)
    p1 = np.ascontiguousarray(np.asarray(p1, dtype=np.float32))
    p2 = np.ascontiguousarray(np.asarray(p2, dtype=np.float32))

    in_maps = []
    for c in range(NCORES):
        sl = slice(c * BC, (c + 1) * BC)
        in_maps.append({"pc": _build_pc(p1[sl], p2[sl]), "wk": wk})

    if _NC_CACHE is None:
        _NC_CACHE = build_bass()
    res = run_bass_kernel_spmd(_NC_CACHE, in_maps, core_ids=list(range(NCORES)))
    out = np.concatenate(
        [_unpack_yraw(res.results[c]["yraw"]) for c in range(NCORES)], axis=0
    )
    return out


# revision 10
# speedup vs baseline: 1.3174x; 1.0505x over previous
"""Trainium2 Bass kernel for nn_BaconAdditionReasoner (histogram_binning).

Math (per batch row):
    P1 = soft_perm(W1), P2 = soft_perm(W2)           (host, 10x10)
    l1 = p1 @ P1.T, l2 = p2 @ P2.T
    u = log(1-l1), v = log(1-l2)
    logprod[k] = sum_{i+j=k} max(u_i, v_j)
              = sum_{i+j=k} u_i + sum_{i+j=k} relu(v_j - u_i)
    e = exp(logprod);  out_k = (e_k - 1) / (sum_k e_k - 19)

Device dataflow (data parallel over 8 cores, 32768 rows/core):
  Front is feature-major: 4 bands of 20 feature rows at 32-aligned
  partitions (PE tile_position needs 32-aligned moving bases), batch on the
  free dim, in half-supertiles of 512 cols (2048 rows); processed in PAIRS
  (1024 cols) so Ln / Exp / the normalization tail amortize their
  per-instruction SBUF/PSUM access cost.
  - L matmul: blockdiag [116->116] f16 matmul per half
  - Ln (ACT): uv = log(1 - l) -> f16, one instr per pair
  - D matmuls: per band, [20->110] f16 pair-diff expansion in PSUM
    (v_j - u_i pairs + -u passthrough rows); 4 PSUM bufs = all bands in
    flight, so one slow relu never starves the PE
  - relu: split across ACT / DVE / Pool per band tile (tunable scheme)
  - A-flip matmuls: per 128-col block, kt [110, 128] is the STATIONARY
    operand (LDWEIGHTS is free) and a [110, 19] +-1 matrix streams as the
    moving operand -> 19 cycles/block; batch-major logprod in PSUM
  - Exp (ACT) -> f32; 19-group reduce / recip / (e-1)*r on DVE per pair
  - output dumped partition-major [128, 4864] f16; host de-interleaves.

All HBM I/O and matmul moving operands are f16 (validated on the real
input distribution: max rel err ~2.7e-3 vs the 2e-2 gate).
"""

import numpy as np

# ---------------------------------------------------------------- constants
B = 262144
NCORES = 8
BC = B // NCORES            # 32768 rows per core
F = 512                     # batch columns per half-supertile (per band)
NB = 4                      # bands (32-aligned partition offsets)
ROWS_H = F * NB             # 2048 rows per half
NH = BC // ROWS_H           # 16 halves per core
NP = NH // 2                # 8 pairs
NCOLS = NH * F              # 8192 columns in pc
KCH = 19 * (ROWS_H // 128)  # 304 output cols per half
KC = 2 * KCH                # 608 per pair
OCOLS = KC * NP             # 4864 output cols

# wk (constants, f16 [128, 256]) column layout
WL0, WL1 = 0, 116           # L blockdiag lhsT [116, 116]
WD0, WD1 = 116, 226         # D pair lhsT [20, 110] replicated per band
WA0, WA1 = 226, 245         # A-flip moving [110, 19]
WKC = 256                   # padded so DMA elem = 512 B

# relu engine per band tile (8 tiles per pair = 2 halves x 4 bands), each a
# list of (engine, c0, c1) over [0, F).
# engines: "A" = ACT (scalar), "D" = DVE (vector), "P" = Pool (gpsimd)
RELU_SCHEME = [
    [("A", 0, 256), ("D", 256, F)], [("D", 0, F)], [("P", 0, F)], [("P", 0, F)],
    [("A", 0, F)], [("D", 0, F)], [("P", 0, F)], [("D", 0, F)],
]


def _soft_perm_np(W: np.ndarray) -> np.ndarray:
    W = W.astype(np.float32)
    lo = W.min(axis=1, keepdims=True)
    hi = W.max(axis=1, keepdims=True)
    Wn = (W - lo) / (hi - lo + np.float32(1e-8))
    return Wn / (Wn.sum(axis=1, keepdims=True) + np.float32(1e-8))


def _build_wk(P1n: np.ndarray, P2n: np.ndarray) -> np.ndarray:
    wk = np.zeros((128, WKC), dtype=np.float32)
    # --- L: lhsT[32q+d, 32q+e] = PP[e, d], PP = blockdiag(P1n, P2n)
    for q in range(NB):
        r = 32 * q
        wk[r : r + 10, r : r + 10] = P1n.T
        wk[r + 10 : r + 20, r + 10 : r + 20] = P2n.T
    # --- D: [20, 110]: pair col 10i+j gets v_j - u_i; col 100+e gets -u_e
    d = np.zeros((20, 110), dtype=np.float32)
    for i in range(10):
        for j in range(10):
            d[i, 10 * i + j] = -1.0
            d[10 + j, 10 * i + j] = 1.0
    for e in range(10):
        d[e, 100 + e] = -1.0
    for q in range(NB):
        wk[32 * q : 32 * q + 20, WD0:WD1] = d
    # --- A-flip moving [110, 19]: pair rows +1 at k=i+j; passthrough rows
    #     (-u values) -1 for k in [e, e+9]
    a = np.zeros((110, 19), dtype=np.float32)
    for i in range(10):
        for j in range(10):
            a[10 * i + j, i + j] = 1.0
    for e in range(10):
        a[100 + e, e : e + 10] = -1.0
    wk[0:110, WA0:WA1] = a
    return wk.astype(np.float16)


def _build_pc(p1c: np.ndarray, p2c: np.ndarray) -> np.ndarray:
    """[BC,10]x2 -> pc [116, NCOLS] f16: row 32q+e = feature e (u: e<10,
    v: 10<=e<20) of band q; col F*g+f = batch row ROWS_H*g + F*q + f."""
    pc = np.zeros((116, NCOLS), dtype=np.float16)
    x1 = p1c.reshape(NH, NB, F, 10)     # [g, q, f, d]
    x2 = p2c.reshape(NH, NB, F, 10)
    for q in range(NB):
        pc[32 * q : 32 * q + 10, :] = (
            x1[:, q].transpose(2, 0, 1).reshape(10, NCOLS).astype(np.float16)
        )
        pc[32 * q + 10 : 32 * q + 20, :] = (
            x2[:, q].transpose(2, 0, 1).reshape(10, NCOLS).astype(np.float16)
        )
    return pc


def _unpack_yraw(yraw: np.ndarray) -> np.ndarray:
    """yraw [128, OCOLS] f16 -> y [BC, 19] f32.
    yraw[p, KC*t + 19*(16h+4q+b) + k] = y[4096t+2048h+512q+128b+p, k]."""
    t = yraw.reshape(128, NP, 2, NB, 4, 19).transpose(1, 2, 3, 4, 0, 5)
    return np.ascontiguousarray(t.reshape(BC, 19).astype(np.float32))


def _patch_act_tables():
    """Force Ln/Exp/Relu to resolve to the single set containing all three
    (natural_log_exp_and_others) so the activation table is loaded once."""
    import concourse.bacc as bacc
    from concourse import mybir

    if getattr(bacc, "_act_tables_patched", False):
        return
    orig = bacc.get_activation_tables
    AF = mybir.ActivationFunctionType
    shared = {AF.Ln, AF.Exp, AF.Relu}

    def patched(arch):
        tabs = orig(arch)
        if "natural_log_exp_and_others" in tabs:
            for name, funcs in tabs.items():
                if name != "natural_log_exp_and_others":
                    tabs[name] = set(funcs) - shared
        return tabs

    bacc.get_activation_tables = patched
    bacc._act_tables_patched = True


def build_bass():
    import concourse.bass as bass
    import concourse.bacc as bacc
    import concourse.tile as tile
    from concourse import mybir

    _patch_act_tables()
    f32 = mybir.dt.float32
    f16 = mybir.dt.float16
    AF = mybir.ActivationFunctionType
    ALU = mybir.AluOpType

    nc = bacc.Bacc("TRN2", target_bir_lowering=False)

    pc_d = nc.dram_tensor("pc", [116, NCOLS], f16, kind="ExternalInput")
    wk_d = nc.dram_tensor("wk", [128, WKC], f16, kind="ExternalInput")
    y_d = nc.dram_tensor("yraw", [128, OCOLS], f16, kind="ExternalOutput")

    with tile.TileContext(nc) as tc:
        with (
            tc.tile_pool(name="singles", bufs=1) as singles,
            tc.tile_pool(name="uv", bufs=2) as uv_p,
            tc.tile_pool(name="kt", bufs=4) as kt_p,
            tc.tile_pool(name="ee", bufs=2) as ee_p,
            tc.tile_pool(name="ss", bufs=2) as ss_p,
            tc.tile_pool(name="rr", bufs=2) as rr_p,
            tc.tile_pool(name="psL", bufs=1, space="PSUM") as psL,
            tc.tile_pool(name="psD", bufs=4, space="PSUM") as psD,
            tc.tile_pool(name="psA", bufs=1, space="PSUM") as psA,
        ):
            wk = singles.tile([128, WKC], f16)
            nc.sync.dma_start(wk[:, :], wk_d[:, :])
            oo = singles.tile([128, OCOLS], f16)

            # prefetch the whole input up front (16 KB/partition), chunked so
            # pair 0 can start after the first quarter lands
            pcc = singles.tile([116, NCOLS], f16)
            for c in range(4):
                c0 = c * (NCOLS // 4)
                nc.sync.dma_start(
                    pcc[:, c0 : c0 + NCOLS // 4], pc_d[:, c0 : c0 + NCOLS // 4]
                )

            def emit_relu(scheme, dp):
                kt = kt_p.tile([110, F], f16)
                for eng, a0, a1 in scheme:
                    if eng == "A":
                        nc.scalar.activation(kt[:, a0:a1], dp[:, a0:a1], AF.Relu)
                    elif eng == "D":
                        nc.vector.tensor_scalar(
                            kt[:, a0:a1], dp[:, a0:a1], 0.0, None, op0=ALU.max
                        )
                    else:
                        nc.gpsimd.tensor_scalar(
                            kt[:, a0:a1], dp[:, a0:a1], 0.0, None, op0=ALU.max
                        )
                return kt

            def emit_tail(pt):
                t, e32 = pt
                ev = e32[:, :].rearrange("p (b k) -> p b k", b=KC // 19, k=19)
                s32 = ss_p.tile([128, KC // 19], f32)
                nc.vector.tensor_reduce(
                    s32[:, :], ev, axis=mybir.AxisListType.X, op=ALU.add
                )
                sm = ss_p.tile([128, KC // 19], f32)
                nc.vector.tensor_scalar(
                    sm[:, :], s32[:, :], -19.0, None, op0=ALU.add
                )
                r32 = rr_p.tile([128, KC // 19], f32)
                nc.vector.reciprocal(r32[:, :], sm[:, :])
                # out = (e - 1) * r  ==  (1-e)/(19-sum(e)), f16
                ov = oo[:, KC * t : KC * (t + 1)].rearrange(
                    "p (b k) -> p b k", b=KC // 19, k=19
                )
                rb = r32[:, :].unsqueeze(-1).broadcast_to([128, KC // 19, 19])
                nc.vector.scalar_tensor_tensor(
                    ov, ev, 1.0, rb, op0=ALU.subtract, op1=ALU.mult
                )
                if t % 2 == 1:
                    o0 = KC * (t - 1)
                    nc.sync.dma_start(
                        y_d[:, o0 : o0 + 2 * KC], oo[:, o0 : o0 + 2 * KC]
                    )

            pending = None
            for t in range(NP):
                base = 2 * F * t
                # l = blockdiag(P1n, P2n) @ p ; one Ln per pair
                lp = psL.tile([116, 2 * F], f32)
                for h in range(2):
                    nc.tensor.matmul(
                        lp[:, F * h : F * (h + 1)], wk[0:116, WL0:WL1],
                        pcc[0:116, base + F * h : base + F * (h + 1)],
                        start=True, stop=True,
                    )
                uvt = uv_p.tile([116, 2 * F], f16)
                nc.scalar.activation(
                    uvt[:, :], lp[:, :], AF.Ln, bias=1.0, scale=-1.0
                )

                ap_t = psA.tile([128, KC], f32)
                for h in range(2):
                    uh = uvt[:, F * h : F * (h + 1)]
                    dps = []
                    for q in range(NB):
                        r = 32 * q
                        dp = psD.tile([110, F], f32)
                        nc.tensor.matmul(
                            dp[:, :], wk[r : r + 20, WD0:WD1], uh[r : r + 20, :],
                            start=True, stop=True, tile_position=(r, 0),
                        )
                        dps.append(dp)
                    kts = [
                        emit_relu(RELU_SCHEME[4 * h + q], dps[q])
                        for q in range(NB)
                    ]
                    # batch-major logprod: kt block stationary, [110,19] moving
                    for q in range(NB):
                        for b in range(F // 128):
                            blk = 16 * h + 4 * q + b
                            nc.tensor.matmul(
                                ap_t[:, 19 * blk : 19 * blk + 19],
                                kts[q][0:110, 128 * b : 128 * b + 128],
                                wk[0:110, WA0:WA1],
                                start=True, stop=True,
                            )

                # e = exp(logprod) (f32: e-1 cancellation needs mantissa)
                e32 = ee_p.tile([128, KC], f32)
                nc.scalar.activation(e32[:, :], ap_t[:, :], AF.Exp)
                # defer this pair's normalization tail until after the NEXT
                # pair's relus so the DVE's in-order stream never lets the
                # tail gate the next pair's grid work
                if pending is not None:
                    emit_tail(pending)
                pending = (t, e32)
            emit_tail(pending)
    nc.compile()
    return nc


_NC_CACHE = None


def kernel(p1, p2, W1, W2):
    global _NC_CACHE
    from concourse.bass_utils import run_bass_kernel_spmd

    P1n = _soft_perm_np(np.asarray(W1))
    P2n = _soft_perm_np(np.asarray(W2))
    wk = _build_wk(P1n, P2n)
    p1 = np.ascontiguousarray(np.asarray(p1, dtype=np.float32))
    p2 = np.ascontiguousarray(np.asarray(p2, dtype=np.float32))

    in_maps = []
    for c in range(NCORES):
        sl = slice(c * BC, (c + 1) * BC)
        in_maps.append({"pc": _build_pc(p1[sl], p2[sl]), "wk": wk})

    if _NC_CACHE is None:
        _NC_CACHE = build_bass()
    res = run_bass_kernel_spmd(_NC_CACHE, in_maps, core_ids=list(range(NCORES)))
    out = np.concatenate(
        [_unpack_yraw(res.results[c]["yraw"]) for c in range(NCORES)], axis=0
    )
    return out
